# revision 14
# baseline (speedup 1.0000x reference)
"""GAT layer kernel for Trainium2 (Bass/Tile), data-parallel over batch on 8 cores.

v16: bf16 masks + bf16 single stationaries (fp8 on DVE/Pool hits a slow
custom-uop path; ACT fp8 ok but PE win not worth it). Mask build split
DVE is_gt (0/1) + ACT Sign (+-1, halved stationaries + folded constants).
Single-bf16 x/W, S_row via V128 broadcast matmul (b folded into exp bias
and mask thresholds), hT+lrelu in the D-tail PE gap, multi-queue x DMA.

Math (per core, N=2048):
    s' = x@W@w_mlp (no bias);  p = exp(s'+b), q = exp(0.2(s'+b))
    M_ij = [s'_i + s'_j + 2b > 0]
    D   = p*(Mp) + q*(Qtot - Mq)
    col = p*(Mr) + q*(Utot - Mu),  r = p/D, u = q/D  (M symmetric)
    out = lrelu(h) * col,  h = x@W
Sign-built blocks use t=+-1 masks with stationary p/2 so PSUM accumulates
(Mp) - P_S/2; the constants fold into the combine tails.
"""

import sys

if "/opt/trn_rl_repo" not in sys.path:
    sys.path.insert(0, "/opt/trn_rl_repo")

from contextlib import ExitStack

import numpy as np

import concourse.bass as bass
import concourse.mybir as mybir
import concourse.tile as tile
from concourse import bacc
from concourse import masks
from concourse.bass_utils import run_bass_kernel_spmd

B, N, F = 8, 2048, 128
NB = N // 128  # 16 token blocks
NC4 = 4  # 512-wide chunks
NEG_SLOPE = 0.2
FP32 = mybir.dt.float32
BF16 = mybir.dt.bfloat16
ALU = mybir.AluOpType
AFT = mybir.ActivationFunctionType

# mask build engine per block: D=vector is_gt 0/1, A=scalar Sign +-1
BLK_ENG = ["D", "D", "D", "A", "D", "D", "D", "A",
           "D", "D", "D", "A", "D", "D", "D", "A"]
A_BLOCKS = [3, 7, 11, 15]  # stride-4 for slicing


def gat_kernel(ctx: ExitStack, tc: "tile.TileContext", out_d, x_d, W_d, wm_d, bm_d):
    nc = tc.nc

    const_p = ctx.enter_context(tc.tile_pool(name="const", bufs=1))
    big_p = ctx.enter_context(tc.tile_pool(name="big", bufs=1))
    mask_p = ctx.enter_context(tc.tile_pool(name="mask", bufs=NB))
    vec_p = ctx.enter_context(tc.tile_pool(name="vec", bufs=1))
    outsb_p = ctx.enter_context(tc.tile_pool(name="outsb", bufs=4))
    # PSUM banks: big=4 (hT / d_ps, sequential), trb=2x1, sm=1
    ps_big = ctx.enter_context(tc.tile_pool(name="ps_big", bufs=1, space="PSUM"))
    ps_tr = ctx.enter_context(tc.tile_pool(name="ps_tr", bufs=3, space="PSUM"))
    ps_sm = ctx.enter_context(tc.tile_pool(name="ps_sm", bufs=1, space="PSUM"))

    # ---------------- input DMAs first (x is the critical path) ----------
    x_view = x_d.rearrange("(t p) f -> p t f", p=128)
    x_sb = big_p.tile([128, NB, 128], FP32, tag="x_sb")
    W_sb = const_p.tile([128, 128], FP32, tag="W_sb")
    wm_sb = const_p.tile([128, 1], FP32, tag="wm_sb")
    b_sb = const_p.tile([1, 1], FP32, tag="b_sb")
    dma_engs = [nc.sync, nc.scalar, nc.gpsimd]
    for t2 in range(8):
        eng = dma_engs[t2 % 3]
        eng.dma_start(x_sb[:, 2 * t2 : 2 * t2 + 2, :], x_view[:, 2 * t2 : 2 * t2 + 2, :])
        if t2 == 0:  # weights after the first x pair: x paces the whole chain
            nc.sync.dma_start(W_sb[:], W_d[:, :])
            nc.scalar.dma_start(wm_sb[:], wm_d.rearrange("(p o) -> p o", o=1))
            nc.scalar.dma_start(b_sb[:], bm_d.rearrange("(p o) -> p o", o=1))

    # ---------------- constants ----------------
    ident_f = const_p.tile([128, 128], FP32, tag="ident_f")
    ident_b = const_p.tile([128, 128], BF16, tag="ident_b")
    masks.make_identity(nc, ident_f[:])
    masks.make_identity(nc, ident_b[:])
    ones_row_f = const_p.tile([1, 128], FP32, tag="ones_row_f")
    nc.gpsimd.memset(ones_row_f[:], 1.0)
    ones_f128 = const_p.tile([128, 128], FP32, tag="ones_f128")
    nc.gpsimd.memset(ones_f128[:], 1.0)
    ones_col = const_p.tile([128, 1], FP32, tag="ones_col")
    nc.gpsimd.memset(ones_col[:], 1.0)

    # Preload the ACT exp table set early (Sign/Copy/Exp share one set)
    warm = const_p.tile([128, 1], FP32, tag="warm")
    nc.scalar.activation(warm[:], ones_col[:], AFT.Exp)

    # b broadcast to [128,1] via K=1 PE matmul
    b_ps = ps_sm.tile([128, 1], FP32, tag="sm")
    nc.tensor.matmul(b_ps[:], lhsT=ones_row_f[:], rhs=b_sb[:], start=True, stop=True)
    b_bc = const_p.tile([128, 1], FP32, tag="b_bc")
    nc.vector.tensor_copy(b_bc[:], b_ps[:])
    b02 = const_p.tile([128, 1], FP32, tag="b02")
    nc.vector.tensor_scalar(b02[:], b_bc[:], NEG_SLOPE, None, ALU.mult)

    # ---------------- x -> bf16, xT via PE transposes ----------------
    x_hi = big_p.tile([128, NB, 128], BF16, tag="x_hi")
    for t2 in range(8):
        sl2 = slice(2 * t2, 2 * t2 + 2)
        if t2 in (5, 7):
            nc.scalar.copy(x_hi[:, sl2, :], x_sb[:, sl2, :])
        else:
            nc.vector.tensor_copy(x_hi[:, sl2, :], x_sb[:, sl2, :])

    xT = big_p.tile([128, N], BF16, tag="xT")  # [f, tok]
    xT_cp = [nc.vector, nc.scalar, nc.vector, nc.scalar,
             nc.vector, nc.scalar, nc.vector, nc.scalar]
    for t2 in range(8):
        tp = ps_tr.tile([128, 256], BF16, tag="trb")
        nc.tensor.matmul(
            tp[:, 0:128], lhsT=x_hi[:, 2 * t2, :], rhs=ident_b[:],
            is_transpose=True, start=True, stop=False,
        )
        nc.tensor.matmul(
            tp[:, 128:256], lhsT=x_hi[:, 2 * t2 + 1, :], rhs=ident_b[:],
            is_transpose=True, start=False, stop=True,
        )
        e = xT_cp[t2]
        if e is nc.scalar:
            e.copy(xT[:, t2 * 256 : (t2 + 1) * 256], tp[:])
        else:
            e.tensor_copy(xT[:, t2 * 256 : (t2 + 1) * 256], tp[:])

    # ---------------- W chain: v = W @ w_mlp, V128, W_hi ----------------
    WT_ps = ps_sm.tile([128, 128], FP32, tag="sm")
    nc.tensor.transpose(WT_ps[:], W_sb[:], ident_f[:])
    WT_sb = vec_p.tile([128, 128], FP32, tag="WT_sb")
    nc.vector.tensor_copy(WT_sb[:], WT_ps[:])
    v_ps = ps_sm.tile([128, 1], FP32, tag="sm")
    nc.tensor.matmul(v_ps[:], lhsT=WT_sb[:], rhs=wm_sb[:], start=True, stop=True)
    v_sb = vec_p.tile([128, 1], FP32, tag="v_sb")
    nc.vector.tensor_copy(v_sb[:], v_ps[:])
    vk = vec_p.tile([128, 1], BF16, tag="vk")
    nc.vector.tensor_copy(vk[:], v_sb[:])
    W_hi = const_p.tile([128, 128], BF16, tag="W_hi")
    nc.scalar.copy(W_hi[:], W_sb[:])
    # V128[:, c] = v for all c (for S_row broadcast matmul)
    V128 = vec_p.tile([128, 128], BF16, tag="V128")
    nc.vector.tensor_scalar(V128[:], ones_f128[:], v_sb[:, 0:1], None, ALU.mult)

    # ---------------- s' [128, 16] via 16 small matmuls (no bias) --------
    s_ps = ps_sm.tile([128, NB], FP32, tag="sm")
    for t in range(NB):
        nc.tensor.matmul(
            s_ps[:, t : t + 1], lhsT=xT[:, t * 128 : (t + 1) * 128], rhs=vk[:],
            start=True, stop=True,
        )
    # thresholds straight from PSUM (no s_mat hop): is_gt blocks use
    # -s'_a - 2b; Sign blocks bias s'_a + 2b
    neg_s = vec_p.tile([128, NB], FP32, tag="neg_s")
    nc.vector.tensor_scalar(neg_s[:], s_ps[:], -1.0, None, ALU.mult)
    nc.vector.tensor_scalar(neg_s[:], neg_s[:], b_bc[:, 0:1], None, ALU.subtract)
    nc.vector.tensor_scalar(neg_s[:], neg_s[:], b_bc[:, 0:1], None, ALU.subtract)
    pos_s2b = vec_p.tile([128, NB], FP32, tag="pos_s2b")
    nc.vector.tensor_scalar(pos_s2b[:], s_ps[:], b_bc[:, 0:1], None, ALU.add)
    nc.vector.tensor_scalar(pos_s2b[:], pos_s2b[:], b_bc[:, 0:1], None, ALU.add)

    # ---------------- S_row [128, 2048] bf16 = s'_n broadcast ------------
    S_row = big_p.tile([128, N], BF16, tag="S_row")
    for c in range(NC4):
        sl = slice(c * 512, (c + 1) * 512)
        S_ps = ps_tr.tile([128, 512], FP32, tag="trb")
        nc.tensor.matmul(S_ps[:], lhsT=V128[:], rhs=xT[:, sl], start=True, stop=True)
        nc.scalar.copy(S_row[:, sl], S_ps[:])

    # ---------------- p, q, bf16 stationaries, constants ----------------
    p_v = vec_p.tile([128, NB], FP32, tag="p_v")
    nc.scalar.activation(p_v[:], s_ps[:], AFT.Exp, bias=b_bc[:, 0:1])
    q_v = vec_p.tile([128, NB], FP32, tag="q_v")
    nc.scalar.activation(q_v[:], s_ps[:], AFT.Exp, scale=NEG_SLOPE, bias=b02[:, 0:1])

    def make_stationary(pv, qv, tagbase):
        """bf16 [128, NB, 2] rows (pv, -qv); A-blocks halved (sign masks)."""
        K = vec_p.tile([128, NB, 2], BF16, tag=tagbase)
        nc.vector.tensor_copy(K[:, :, 0], pv[:])
        nc.vector.tensor_scalar(K[:, :, 1], qv[:], -1.0, None, ALU.mult)
        nc.vector.tensor_scalar(K[:, 3::4, 0], pv[:, 3::4], 0.5, None, ALU.mult)
        nc.vector.tensor_scalar(K[:, 3::4, 1], qv[:, 3::4], -0.5, None, ALU.mult)
        return K

    def make_consts(pv, qv, tagbase):
        """c_p = P_A/2 bcast, c_q = TOT_q - Q_A/2 bcast  ([128,1] each)."""
        cin = vec_p.tile([128, 3], FP32, tag=tagbase + "_in")
        nc.vector.reduce_sum(cin[:, 0:1], qv[:], axis=mybir.AxisListType.X)
        nc.vector.reduce_sum(cin[:, 1:2], pv[:, 3::4], axis=mybir.AxisListType.X)
        nc.vector.reduce_sum(cin[:, 2:3], qv[:, 3::4], axis=mybir.AxisListType.X)
        cps = ps_sm.tile([128, 3], FP32, tag="sm")
        nc.tensor.matmul(cps[:], lhsT=ones_f128[:], rhs=cin[:], start=True, stop=True)
        cbc = vec_p.tile([128, 3], FP32, tag=tagbase + "_bc")
        nc.vector.tensor_copy(cbc[:], cps[:])
        c_p = vec_p.tile([128, 1], FP32, tag=tagbase + "_cp")
        nc.vector.tensor_scalar(c_p[:], cbc[:, 1:2], 0.5, None, ALU.mult)
        c_q = vec_p.tile([128, 1], FP32, tag=tagbase + "_cq")
        nc.vector.tensor_scalar(c_q[:], cbc[:, 2:3], -0.5, None, ALU.mult)
        nc.vector.tensor_tensor(c_q[:], c_q[:], cbc[:, 0:1], ALU.add)
        return c_p, c_q

    Pk = make_stationary(p_v, q_v, "Pk")
    c1, c2 = make_consts(p_v, q_v, "k1")
    pc1 = vec_p.tile([128, NB], FP32, tag="pc1")
    pc1b = vec_p.tile([128, NB], FP32, tag="pc1b")
    nc.vector.tensor_scalar(pc1[:], p_v[:], c1[:, 0:1], None, ALU.mult)
    nc.vector.tensor_scalar(pc1b[:], q_v[:], c2[:, 0:1], None, ALU.mult)
    nc.vector.tensor_tensor(pc1[:], pc1[:], pc1b[:], ALU.add)

    # ---------------- hT chunks 0-2 + relu piece (pre-mask, via ps_tr) ----
    # lrelu(x) = 0.2*x + 0.8*relu(x). STT combines run post-mv1 so Pk/masks
    # aren't delayed on DVE; the held ps_tr tiles are unused during mv1.
    rel08 = big_p.tile([128, N], BF16, tag="rel08")
    lrlT = big_p.tile([128, N], BF16, tag="lrlT")
    h_tiles = []
    for c in range(3):
        sl = slice(c * 512, (c + 1) * 512)
        h_ps = ps_tr.tile([128, 512], FP32, tag="trb")
        nc.tensor.matmul(h_ps[:], lhsT=W_hi[:], rhs=xT[:, sl], start=True, stop=True)
        nc.scalar.activation(rel08[:, sl], h_ps[:], AFT.Relu, scale=0.8)
        h_tiles.append(h_ps)

    # ---------------- masks: bf16, DVE is_gt + ACT Sign ----------------
    m_tiles = []
    for _a in range(NB):
        m_t = mask_p.tile([128, N], BF16, tag="mask")
        m_tiles.append(m_t)
    for a in range(NB):
        if BLK_ENG[a] == "A":
            nc.scalar.activation(
                m_tiles[a][:, :], S_row[:, :], AFT.Sign,
                bias=pos_s2b[:, a : a + 1],
            )
        else:
            nc.vector.tensor_scalar(
                m_tiles[a][:, :], S_row[:, :], neg_s[:, a : a + 1], None,
                ALU.is_gt,
            )

    # ---------------- matvec 1 (block-outer) ----------------
    d_ps = ps_big.tile([2, N], FP32, tag="bigps")
    for a in range(NB):
        for c in range(NC4):
            nc.tensor.matmul(
                d_ps[:, c * 512 : (c + 1) * 512],
                lhsT=Pk[:, a, :],
                rhs=m_tiles[a][:, c * 512 : (c + 1) * 512],
                start=(a == 0),
                stop=(a == NB - 1),
            )

    # ---------------- hT chunk 3 + lrelu combines (post-mv1) ----------
    sl3 = slice(3 * 512, 4 * 512)
    h_ps3 = ps_tr.tile([128, 512], FP32, tag="trb")
    nc.tensor.matmul(h_ps3[:], lhsT=W_hi[:], rhs=xT[:, sl3], start=True, stop=True)
    nc.scalar.activation(rel08[:, sl3], h_ps3[:], AFT.Relu, scale=0.8)
    h_tiles.append(h_ps3)
    for c in range(NC4):
        sl = slice(c * 512, (c + 1) * 512)
        nc.vector.scalar_tensor_tensor(
            lrlT[:, sl], h_tiles[c][:], NEG_SLOPE, rel08[:, sl], ALU.mult, ALU.add
        )

    # ---------------- D tail: transpose + combine ----------------
    d_sb = vec_p.tile([2, N], FP32, tag="d_sb")
    d_cp = [nc.vector, nc.scalar, nc.vector, nc.scalar]
    for c in range(NC4):
        sl = slice(c * 512, (c + 1) * 512)
        e = d_cp[c]
        if e is nc.scalar:
            e.copy(d_sb[:, sl], d_ps[:, sl])
        else:
            e.tensor_copy(d_sb[:, sl], d_ps[:, sl])
    Dp = vec_p.tile([128, NB, 2], FP32, tag="Dp")
    for c in range(NC4):
        dtp = ps_tr.tile([128, 8], FP32, tag="trb")
        for t4 in range(4):
            t = c * 4 + t4
            nc.tensor.matmul(
                dtp[:, 2 * t4 : 2 * t4 + 2],
                lhsT=d_sb[:, t * 128 : (t + 1) * 128],
                rhs=ident_f[0:2, 0:2],
                is_transpose=True, start=(t4 == 0), stop=(t4 == 3),
            )
        if c % 2 == 0:
            nc.vector.tensor_copy(Dp[:, c * 4 : c * 4 + 4, :], dtp[:])
        else:
            nc.scalar.copy(Dp[:, c * 4 : c * 4 + 4, :], dtp[:])

    # D = p*row0 + q*row1 + pc1   (pc1 = p*c1 + q*c2, precomputed)
    t1 = vec_p.tile([128, NB], FP32, tag="t1")
    nc.vector.tensor_tensor(t1[:], p_v[:], Dp[:, :, 0], ALU.mult)
    t2 = vec_p.tile([128, NB], FP32, tag="t2")
    nc.vector.tensor_tensor(t2[:], q_v[:], Dp[:, :, 1], ALU.mult)
    D_v = vec_p.tile([128, NB], FP32, tag="D_v")
    nc.vector.tensor_tensor(D_v[:], t1[:], t2[:], ALU.add)
    nc.vector.tensor_tensor(D_v[:], D_v[:], pc1[:], ALU.add)
    invD = vec_p.tile([128, NB], FP32, tag="invD")
    nc.vector.reciprocal(invD[:], D_v[:])
    rr = vec_p.tile([128, NB], FP32, tag="rr")
    nc.vector.tensor_tensor(rr[:], p_v[:], invD[:], ALU.mult)
    uu = vec_p.tile([128, NB], FP32, tag="uu")
    nc.vector.tensor_tensor(uu[:], q_v[:], invD[:], ALU.mult)
    Rk = make_stationary(rr, uu, "Rk")
    c3, c4 = make_consts(rr, uu, "k2")
    pc2 = vec_p.tile([128, NB], FP32, tag="pc2")
    pc2b = vec_p.tile([128, NB], FP32, tag="pc2b")
    nc.vector.tensor_scalar(pc2[:], p_v[:], c3[:, 0:1], None, ALU.mult)
    nc.vector.tensor_scalar(pc2b[:], q_v[:], c4[:, 0:1], None, ALU.mult)
    nc.vector.tensor_tensor(pc2[:], pc2[:], pc2b[:], ALU.add)

    # ---------------- lrelu transposes (PE, pre-mv2) ----------------
    lrl_sb = big_p.tile([128, NB, 128], BF16, tag="lrl_sb")
    lrl_cp = [nc.vector, nc.scalar, nc.vector, nc.scalar,
              nc.vector, nc.scalar, nc.vector, nc.scalar]

    def emit_lrl_transposes():
        for t2 in range(8):
            op = ps_tr.tile([128, 256], BF16, tag="trb")
            nc.tensor.matmul(
                op[:, 0:128], lhsT=lrlT[:, 256 * t2 : 256 * t2 + 128], rhs=ident_b[:],
                is_transpose=True, start=True, stop=False,
            )
            nc.tensor.matmul(
                op[:, 128:256], lhsT=lrlT[:, 256 * t2 + 128 : 256 * t2 + 256],
                rhs=ident_b[:], is_transpose=True, start=False, stop=True,
            )
            e = lrl_cp[t2]
            if e is nc.scalar:
                e.copy(lrl_sb[:, 2 * t2 : 2 * t2 + 2, :], op[:])
            else:
                e.tensor_copy(lrl_sb[:, 2 * t2 : 2 * t2 + 2, :], op[:])

    # ---------------- matvec 2 (c-outer) + pipelined tails ----------
    out_view = out_d.rearrange("(t p) f -> p t f", p=128)
    g_sb = vec_p.tile([2, N], FP32, tag="g_sb")
    Gp = vec_p.tile([128, NB, 2], FP32, tag="Gp")
    colf = vec_p.tile([128, NB], FP32, tag="colf")
    ga = vec_p.tile([128, NB], FP32, tag="ga")
    gb = vec_p.tile([128, NB], FP32, tag="gb")

    def mv2_group(t0, nblk):
        sl = slice(t0 * 128, (t0 + nblk) * 128)
        g_ps = ps_tr.tile([2, nblk * 128], FP32, tag="trb")
        for a in range(NB):
            nc.tensor.matmul(
                g_ps[:],
                lhsT=Rk[:, a, :],
                rhs=m_tiles[a][:, sl],
                start=(a == 0),
                stop=(a == NB - 1),
            )
        return g_ps

    def mv2_tail(t0, nblk, even, g_ps):
        sl = slice(t0 * 128, (t0 + nblk) * 128)
        tsl = slice(t0, t0 + nblk)
        if even:
            nc.vector.tensor_copy(g_sb[:, sl], g_ps[:])
        else:
            nc.scalar.copy(g_sb[:, sl], g_ps[:])
        gtp = ps_tr.tile([128, 2 * nblk], FP32, tag="trb")
        for t4 in range(nblk):
            t = t0 + t4
            nc.tensor.matmul(
                gtp[:, 2 * t4 : 2 * t4 + 2],
                lhsT=g_sb[:, t * 128 : (t + 1) * 128],
                rhs=ident_f[0:2, 0:2],
                is_transpose=True, start=(t4 == 0), stop=(t4 == nblk - 1),
            )
        if even:
            nc.vector.tensor_copy(Gp[:, tsl, :], gtp[:])
        else:
            nc.scalar.copy(Gp[:, tsl, :], gtp[:])
        # col = p*G0 + q*G1 + pc2   (pc2 = p*c3 + q*c4, precomputed)
        nc.vector.tensor_tensor(ga[:, tsl], p_v[:, tsl], Gp[:, tsl, 0], ALU.mult)
        nc.vector.tensor_tensor(gb[:, tsl], q_v[:, tsl], Gp[:, tsl, 1], ALU.mult)
        nc.vector.tensor_tensor(colf[:, tsl], ga[:, tsl], gb[:, tsl], ALU.add)
        nc.vector.tensor_tensor(colf[:, tsl], colf[:, tsl], pc2[:, tsl], ALU.add)
        for t4 in range(nblk):
            t = t0 + t4
            o_sb = outsb_p.tile([128, 128], FP32, tag="o_sb")
            if t % 2 == 0:
                nc.vector.tensor_scalar(
                    o_sb[:], lrl_sb[:, t, :], colf[:, t : t + 1], None, ALU.mult
                )
            else:
                nc.scalar.activation(
                    o_sb[:], lrl_sb[:, t, :], AFT.Copy, scale=colf[:, t : t + 1]
                )
            (nc.sync if t % 2 == 0 else nc.gpsimd).dma_start(
                out_view[:, t, :], o_sb[:]
            )

    emit_lrl_transposes()
    # groups: 3x512 cols + 2x256: the final tail (serial end-of-kernel) halves
    groups = [(0, 4, True), (4, 4, False), (8, 4, True), (12, 2, False), (14, 2, True)]
    pending = None
    for t0, nblk, even in groups:
        g_ps_c = mv2_group(t0, nblk)
        if pending is not None:
            mv2_tail(pending[0], pending[1], pending[2], pending[3])
        pending = (t0, nblk, even, g_ps_c)
    mv2_tail(pending[0], pending[1], pending[2], pending[3])


def build_nc(num_devices: int = 8) -> "bass.Bass":
    nc = bacc.Bacc(
        "TRN2", target_bir_lowering=False, debug=False, num_devices=num_devices
    )
    x_d = nc.dram_tensor("x", [N, F], FP32, kind="ExternalInput")
    W_d = nc.dram_tensor("W", [F, F], FP32, kind="ExternalInput")
    wm_d = nc.dram_tensor("w_mlp", [F], FP32, kind="ExternalInput")
    bm_d = nc.dram_tensor("b_mlp", [1], FP32, kind="ExternalInput")
    out_d = nc.dram_tensor("out", [N, F], FP32, kind="ExternalOutput")
    with tile.TileContext(nc) as tc:
        with ExitStack() as ctx:
            gat_kernel(ctx, tc, out_d.ap(), x_d.ap(), W_d.ap(), wm_d.ap(), bm_d.ap())
    nc.compile()
    return nc


_NC_CACHE: dict = {}


def run(x, W, w_mlp, b_mlp, trace=False, **spmd_kwargs):
    x = np.asarray(x, dtype=np.float32)
    W = np.asarray(W, dtype=np.float32)
    w_mlp = np.asarray(w_mlp, dtype=np.float32)
    b_mlp = np.asarray(b_mlp, dtype=np.float32)

    if "nc" not in _NC_CACHE:
        _NC_CACHE["nc"] = build_nc(num_devices=B)
    nc = _NC_CACHE["nc"]

    in_maps = [
        {"x": np.ascontiguousarray(x[b, 0]), "W": W, "w_mlp": w_mlp, "b_mlp": b_mlp}
        for b in range(B)
    ]
    res = run_bass_kernel_spmd(
        nc, in_maps, core_ids=list(range(B)), trace=trace, **spmd_kwargs
    )
    out = np.stack([res.results[b]["out"] for b in range(B)])[:, None]
    return out.astype(np.float32), res


def kernel(x, W, w_mlp, b_mlp):
    out, _ = run(x, W, w_mlp, b_mlp)
    return out
